# revision 1
# baseline (speedup 1.0000x reference)
"""Trainium2 Bass kernel for nn_MiniGRUConv2d4 (MinGRU 4-direction conv scan).

Problem (B=4, Cin=64, Cout4=256, H=W=256):
    u_c  = conv3x3(xs, w_c) + bn_c          for c in {z, h, s}   (Cout=256)
    z    = sigmoid(u_z); hh = u_h; s = sigmoid(u_s)
    split 256 channels into 4 groups of 64; group g scans
      g=0: over H fwd, g=1: over H rev, g=2: over W fwd, g=3: over W rev
      h_i = z_i*hh_i + (1-z_i)*h_{i-1}
    out  = sum_g s_g * h_g                  (B, 64, H, W)

Sharding (8 cores): core = (batch b, orientation o).
  o=0: natural image, conv channels 128..255 (groups 2,3: W-fwd / W-rev)
  o=1: transposed image (host transposes), channels 0..127 (groups 0,1:
       H-scan becomes W-scan in the transposed frame).
Every core runs the identical program; group A (partitions 0:64) scans
forward along W, group B (partitions 64:128) scans backward (implemented
with negative-stride APs feeding the hardware scan instruction). The conv
is 6 K=128 fp32r matmuls per 2-row x 256-col tile (3x3 taps: dy0/dy1 pairs
packed into the 128-partition contraction via a row-shifted second copy of
the input; dy2 row rides in the upper 64 partitions with zero top-half
weights). Host folds BN into weights/biases, pads, transposes, and sums the
per-core partial products.
"""

import sys
import types

import numpy as np

import concourse.bass as bass
import concourse.mybir as mybir
import concourse.tile as tile

F32 = mybir.dt.float32
F32R = mybir.dt.float32r
AF = mybir.ActivationFunctionType
OP = mybir.AluOpType

_R = 8  # band height (output rows per band); W*R*4B per partition band tiles


# ---------------------------------------------------------------------------
# Workaround: the pinned walrus rejects instructions carrying more than a
# couple of sem waits ("Too many sync wait commands", CoreV3GenImpl
# setupSyncWait). Hoist excess waits onto same-engine NOPs inserted right
# before the offending instruction.
_MAX_WAITS = 1


def _split_excess_waits(nc, max_waits=_MAX_WAITS):
    import bass_rust

    n_split = 0
    for f in nc.m.functions:
        for blk in f.blocks:
            out = []
            for inst in blk.instructions:
                si = inst.sync_info
                if si is not None and len(si.on_wait) > max_waits:
                    waits = list(si.on_wait)
                    extra, keep = waits[:-max_waits], waits[-max_waits:]
                    for i0 in range(0, len(extra), max_waits):
                        nop = mybir.InstNoOp(
                            name=f"{inst.name}_xw{i0}", ins=[], outs=[]
                        )
                        nop.engine = inst.engine
                        nop.sync_info = bass_rust.SyncInfo(
                            on_wait=extra[i0 : i0 + max_waits], on_update=[]
                        )
                        nc.register_instruction(nop)
                        out.append(nop)
                        n_split += 1
                    inst.sync_info = bass_rust.SyncInfo(
                        on_wait=keep, on_update=list(si.on_update)
                    )
                out.append(inst)
            blk.instructions = out
    return n_split


def _ensure_axon_hooks_importable():
    # bass_utils imports antenv.axon_hooks when tracing is requested; the
    # container's antenv stub lacks it. Provide a no-op registry so the
    # import never crashes (tracing then just degrades gracefully).
    try:
        import antenv.axon_hooks  # noqa: F401
    except Exception:
        try:
            import antenv

            mod = types.ModuleType("antenv.axon_hooks")
            mod._hook = None
            mod.set_axon_ntff_profile_hook = lambda h: setattr(mod, "_hook", h)
            mod.get_axon_ntff_profile_hook = lambda: mod._hook
            sys.modules["antenv.axon_hooks"] = mod
            antenv.axon_hooks = mod
        except Exception:
            pass


# ---------------------------------------------------------------------------
# Device program


# Conv operands: fp16 runs the PE at full rate (1 cyc/row, like bf16) but
# carries a 10-bit mantissa — conv error ~5e-4 vs bf16's ~3e-3. All values
# here are O(1-10), far from fp16 range limits. fp32r would be exact-ish but
# its fused 4-byte weight load can't pipeline (measured +50 ns/MM).
CONV_DT = mybir.dt.float16
CHAIN_DT = mybir.dt.float16  # z/s/a/b/h/p tiles + output (host upcasts)
# GpSimd shares an SBUF port pair with VectorE (exclusive lock): running bulk
# elementwise there slows every concurrent DVE op ~20%. Keep GpSimd idle.
P_ON_POOL = False
FIXUPS_ON_POOL = False  # pinned walrus: TensorScalarPtr not legal on Pool
MEMSET_ON_ACT = True
SPLIT_SCAN = True  # row-split scan via SBUF->SBUF DMA remap (latency-bound)


def build_nc(H, W, with_init_fixup=True):
    """One-core program; all 8 cores run it SPMD with different inputs."""
    R = _R
    RR = R + 1  # input rows resident per band (dy0/dy1 buffer)
    Wp = W + 2
    assert H % R == 0 and W % 2 == 0
    nbands = H // R
    cdt = CONV_DT
    wdt = CHAIN_DT

    nc = bass.Bass("TRN2", target_bir_lowering=False, debug=False)
    xp = nc.dram_tensor("xp", [64, H + 2, Wp], cdt, kind="ExternalInput").ap()
    wts = nc.dram_tensor("wts", [128, 15, 128], cdt, kind="ExternalInput").ap()
    consts = nc.dram_tensor("consts", [128, 4], F32, kind="ExternalInput").ap()
    out = nc.dram_tensor("out", [128, H * W], wdt, kind="ExternalOutput").ap()

    with tile.TileContext(nc) as tc:
        with (
            tc.tile_pool(name="const", bufs=1) as cpool,
            tc.tile_pool(name="xin", bufs=3) as xpool,
            tc.tile_pool(name="work", bufs=3) as wpool,
            tc.tile_pool(name="psum", bufs=2, space="PSUM") as ppool,
        ):
            wts_sb = cpool.tile([128, 15, 128], cdt)
            nc.sync.dma_start(wts_sb[:], wts)
            cst = cpool.tile([128, 4], F32)
            nc.sync.dma_start(cst[:], consts)
            bias = [cst[:, c : c + 1] for c in range(3)]  # z, h, s
            init = cst[:, 3:4]

            for band in range(nbands):
                y0 = band * R
                # x2: dy0 rows at partitions 0:64, dy1 rows at 64:128
                x2 = xpool.tile([128, RR, Wp], cdt)
                nc.sync.dma_start(x2[0:64], xp[:, y0 : y0 + RR, :])
                nc.sync.dma_start(x2[64:128], xp[:, y0 + 1 : y0 + 1 + RR, :])
                # x3: dy2 rows; lower = col+0, upper = col+1. Upper's last
                # column is never loaded; zero it so the K=128 dx2 stream
                # (zero top weights) can't hit NaN garbage.
                x3 = xpool.tile([128, R, Wp], cdt)
                nc.sync.dma_start(x3[0:64], xp[:, y0 + 2 : y0 + 2 + R, :])
                nc.vector.memset(x3[64:128, :, Wp - 1 : Wp], 0.0)
                nc.sync.dma_start(
                    x3[64:128, :, 0 : Wp - 1], xp[:, y0 + 2 : y0 + 2 + R, 1:Wp]
                )

                z_b = wpool.tile([128, R * W], wdt)
                s_b = wpool.tile([128, R * W], wdt)
                # a and b share one tile: [a (R*W) | b (R*W)] so one DMA
                # descriptor pair covers both in the row-split remap below
                ab = wpool.tile([128, 2 * R * W], wdt)
                a_b = ab[:, 0 : R * W]
                b_b = ab[:, R * W : 2 * R * W]
                h_b = wpool.tile([128, R * W], wdt)
                p_b = wpool.tile([128, R * W], wdt)
                if SPLIT_SCAN:
                    # row-split scan operands: partitions = (row-half, chan),
                    # free = [a | b] x (R/2 rows x W). Halves scan wall time
                    # (the hw scan is ~2 cyc/elem and partition-count-free).
                    absA = wpool.tile([128, R * W], wdt)
                    absB = wpool.tile([128, R * W], wdt)
                    hsA = wpool.tile([128, R * W // 2], wdt)
                    hsB = wpool.tile([128, R * W // 2], wdt)

                for j0 in range(0, R, 2):  # unit: 2 output rows = N=512
                    us = []
                    for c in range(3):  # z, h, s convs
                        u = ppool.tile(
                            [128, 2 * W], F32, name=f"u{c}", tag=f"u{c}",
                            bufs=(4 if c == 1 else 2),
                        )
                        for dx in range(3):  # dy0+dy1 pairs
                            nc.tensor.matmul(
                                u[:],
                                wts_sb[:, 3 * c + dx, :],
                                x2[:, j0 : j0 + 2, dx : dx + W],
                                start=(dx == 0),
                                stop=False,
                            )
                        # dy2 (dx0, dx1) pair
                        nc.tensor.matmul(
                            u[:],
                            wts_sb[:, 9 + c, :],
                            x3[:, j0 : j0 + 2, 0:W],
                            start=False,
                            stop=False,
                        )
                        # dy2 dx2 (upper weights zero -> K=128 uniform)
                        nc.tensor.matmul(
                            u[:],
                            wts_sb[:, 12 + c, :],
                            x3[:, j0 : j0 + 2, 2 : 2 + W],
                            start=False,
                            stop=True,
                        )
                        us.append(u)
                    sl = slice(j0 * W, (j0 + 2) * W)
                    nc.scalar.activation(z_b[:, sl], us[0][:], AF.Sigmoid, bias=bias[0])
                    nc.scalar.activation(s_b[:, sl], us[2][:], AF.Sigmoid, bias=bias[2])
                    # b = (u_h + bias_h) * z
                    nc.vector.scalar_tensor_tensor(
                        b_b[:, sl], us[1][:], bias[1], z_b[:, sl], op0=OP.add, op1=OP.mult
                    )

                # a = 1 - z (on ACT: Identity(-z + 1); DVE is the scarce engine)
                nc.scalar.activation(
                    a_b[:], z_b[:], AF.Identity, bias=1.0, scale=-1.0
                )
                a3 = a_b.rearrange("p (r w) -> p r w", w=W)
                b3 = b_b.rearrange("p (r w) -> p r w", w=W)
                # fold the (normally zero) scan init into b at each row edge,
                # then zero `a` there so the flat scan restarts per row.
                if with_init_fixup:
                    nc.vector.scalar_tensor_tensor(
                        b3[0:64, :, 0], a3[0:64, :, 0], init[0:64], b3[0:64, :, 0],
                        op0=OP.mult, op1=OP.add,
                    )
                    nc.vector.scalar_tensor_tensor(
                        b3[64:128, :, W - 1], a3[64:128, :, W - 1], init[64:128],
                        b3[64:128, :, W - 1], op0=OP.mult, op1=OP.add,
                    )
                if MEMSET_ON_ACT:
                    nc.scalar.activation(
                        a3[0:64, :, 0], a3[0:64, :, 0], AF.Copy, bias=0.0, scale=0.0
                    )
                    nc.scalar.activation(
                        a3[64:128, :, W - 1], a3[64:128, :, W - 1], AF.Copy,
                        bias=0.0, scale=0.0,
                    )
                else:
                    nc.vector.memset(a3[0:64, :, 0], 0.0)
                    nc.vector.memset(a3[64:128, :, W - 1], 0.0)
                if SPLIT_SCAN:
                    # remap a|b into row-split layout (SBUF->SBUF DMA, off
                    # the DVE critical path), scan at full 128-partition
                    # width, then remap h back.
                    Rh = R // 2
                    ab4 = ab.rearrange("p (pl r w) -> p pl r w", pl=2, w=W)
                    absA4 = absA.rearrange("p (pl r w) -> p pl r w", pl=2, w=W)
                    absB4 = absB.rearrange("p (pl r w) -> p pl r w", pl=2, w=W)
                    nc.sync.dma_start(absA4[0:64], ab4[0:64, :, 0:Rh, :])
                    nc.sync.dma_start(absA4[64:128], ab4[0:64, :, Rh:R, :])
                    nc.sync.dma_start(absB4[0:64], ab4[64:128, :, 0:Rh, :])
                    nc.sync.dma_start(absB4[64:128], ab4[64:128, :, Rh:R, :])
                    half = R * W // 2
                    nc.vector.tensor_tensor_scan(
                        hsA[:, :], absA[:, 0:half], absA[:, half : 2 * half],
                        0.0, op0=OP.mult, op1=OP.add,
                    )
                    nc.vector.tensor_tensor_scan(
                        hsB[:, ::-1], absB[:, 0:half][:, ::-1],
                        absB[:, half : 2 * half][:, ::-1], 0.0,
                        op0=OP.mult, op1=OP.add,
                    )
                    h3 = h_b.rearrange("p (r w) -> p r w", w=W)
                    hsA3 = hsA.rearrange("p (r w) -> p r w", w=W)
                    hsB3 = hsB.rearrange("p (r w) -> p r w", w=W)
                    nc.sync.dma_start(h3[0:64, 0:Rh, :], hsA3[0:64])
                    nc.sync.dma_start(h3[0:64, Rh:R, :], hsA3[64:128])
                    nc.sync.dma_start(h3[64:128, 0:Rh, :], hsB3[0:64])
                    nc.sync.dma_start(h3[64:128, Rh:R, :], hsB3[64:128])
                else:
                    # group A fwd; group B backward via reversed APs
                    nc.vector.tensor_tensor_scan(
                        h_b[0:64, :], a_b[0:64, :], b_b[0:64, :], 0.0,
                        op0=OP.mult, op1=OP.add,
                    )
                    nc.vector.tensor_tensor_scan(
                        h_b[64:128, ::-1], a_b[64:128, ::-1],
                        b_b[64:128, ::-1], 0.0, op0=OP.mult, op1=OP.add,
                    )
                p_eng = nc.gpsimd if P_ON_POOL else nc.vector
                p_eng.tensor_mul(p_b[:], s_b[:], h_b[:])
                nc.sync.dma_start(out[:, y0 * W : (y0 + R) * W], p_b[:])
    _split_excess_waits(nc)
    return nc


# ---------------------------------------------------------------------------
# Host side

_NC_CACHE = {}


def _get_nc(H, W, with_init_fixup=True):
    key = (H, W, with_init_fixup)
    if key not in _NC_CACHE:
        _NC_CACHE[key] = build_nc(H, W, with_init_fixup)
    return _NC_CACHE[key]


def make_in_maps(inputs, H, W):
    """Build the 8 per-core input dicts from the full problem inputs."""
    xs = np.ascontiguousarray(np.asarray(inputs["xs"], dtype=np.float32))
    B = xs.shape[0]
    Ws, Bs = {}, {}
    for tag in ("z", "h", "s"):
        w = np.asarray(inputs["w_" + tag], dtype=np.float32)
        g = np.asarray(inputs["g_" + tag], dtype=np.float32)
        be = np.asarray(inputs["b_" + tag], dtype=np.float32)
        m = np.asarray(inputs["m_" + tag], dtype=np.float32)
        v = np.asarray(inputs["v_" + tag], dtype=np.float32)
        inv = g / np.sqrt(v + 1e-5)
        Ws[tag] = w * inv[:, None, None, None]
        Bs[tag] = be - m * inv
    init = {
        k: np.asarray(inputs[k], dtype=np.float32).reshape(-1)
        for k in ("h20", "h21", "h30", "h31")
    }

    in_maps = []
    for b in range(B):
        for orient in (0, 1):
            if orient == 0:
                img = xs[b]
                ch = slice(128, 256)
                init_a, init_b = init["h30"], init["h31"]
            else:
                img = xs[b].transpose(0, 2, 1)
                ch = slice(0, 128)
                init_a, init_b = init["h20"], init["h21"]
            xpad = np.pad(img, ((0, 0), (1, 1), (1, 1)))
            wts = np.zeros((128, 15, 128), np.float32)
            consts = np.zeros((128, 4), np.float32)
            for c, tag in enumerate(("z", "h", "s")):
                wc = Ws[tag][ch]  # (128, 64, 3, 3) [cout, cin, ky, kx]
                if orient == 1:
                    wc = wc.transpose(0, 1, 3, 2)
                for dx in range(3):
                    wts[0:64, 3 * c + dx, :] = wc[:, :, 0, dx].T
                    wts[64:128, 3 * c + dx, :] = wc[:, :, 1, dx].T
                wts[0:64, 9 + c, :] = wc[:, :, 2, 0].T
                wts[64:128, 9 + c, :] = wc[:, :, 2, 1].T
                wts[0:64, 12 + c, :] = wc[:, :, 2, 2].T
                consts[:, c] = Bs[tag][ch]
            consts[0:64, 3] = init_a
            consts[64:128, 3] = init_b
            cnp = mybir.dt.np(CONV_DT)
            if xpad.dtype != cnp:
                xpad = xpad.astype(cnp)
                wts = wts.astype(cnp)
            in_maps.append(
                {
                    "xp": np.ascontiguousarray(xpad),
                    "wts": wts,
                    "consts": consts,
                }
            )
    return in_maps


def gather_output(core_outs, B, H, W):
    """core_outs: list of 8 arrays (128, H*W) in core order (b-major)."""
    out = np.empty((B, 64, H, W), np.float32)
    for b in range(B):
        nat = core_outs[2 * b].astype(np.float32).reshape(2, 64, H, W)
        tr = core_outs[2 * b + 1].astype(np.float32).reshape(2, 64, W, H)
        out[b] = nat[0] + nat[1] + (tr[0] + tr[1]).transpose(0, 2, 1)
    return out


def kernel(**inputs):
    from concourse.bass_utils import run_bass_kernel_spmd

    _ensure_axon_hooks_importable()
    xs = inputs["xs"]
    B, C, H, W = xs.shape
    # the scan-init fixup ops are only needed for nonzero initial states
    # (the problem spec ships all-zero inits)
    need_fixup = any(
        np.any(np.asarray(inputs[k], dtype=np.float32))
        for k in ("h20", "h21", "h30", "h31")
    )
    nc = _get_nc(H, W, with_init_fixup=need_fixup)
    in_maps = make_in_maps(inputs, H, W)
    res = run_bass_kernel_spmd(nc, in_maps, core_ids=list(range(len(in_maps))))
    outs = [res.results[c]["out"] for c in range(len(in_maps))]
    return gather_output(outs, B, H, W)



# revision 3
# speedup vs baseline: 1.4254x; 1.4254x over previous
"""Trainium2 Bass kernel for nn_MiniGRUConv2d4 (MinGRU 4-direction conv scan).

Problem (B=4, Cin=64, Cout4=256, H=W=256):
    u_c  = conv3x3(xs, w_c) + bn_c          for c in {z, h, s}   (Cout=256)
    z    = sigmoid(u_z); hh = u_h; s = sigmoid(u_s)
    split 256 channels into 4 groups of 64; group g scans
      g=0: over H fwd, g=1: over H rev, g=2: over W fwd, g=3: over W rev
      h_i = z_i*hh_i + (1-z_i)*h_{i-1}
    out  = sum_g s_g * h_g                  (B, 64, H, W)

Sharding (8 cores): core = (batch b, orientation o).
  o=0: natural image, conv channels 128..255 (groups 2,3: W-fwd / W-rev)
  o=1: transposed image (host transposes), channels 0..127 (groups 0,1:
       H-scan becomes W-scan in the transposed frame).

v2 layout: each conv PSUM tile holds ONE scan group (64 chans) in
row-split form — partitions = (row-half h, chan c), filled by two
concurrent M=64 matmuls (col-group tiling: tile_position (0,0) and
(0,64)) whose rhs streams come from different band rows. The scan then
runs at full 128-partition width directly on conv output; the v1
SBUF->SBUF DMA remap (2 MB/band, 62% of all DMA traffic, and the sync
-queue head-of-line blocker) is gone. The conv itself is 5 K=128 fp16
matmul waves per (j, conv, group) tile (3x3 taps: dy0/dy1 pairs packed
into the 128-partition contraction via a row-shifted second copy of the
input; dy2 row rides partition-packed dx0/dx1 + a zero-top-half dx2
slot). Host folds BN into weights/biases, pads, transposes, and sums
the per-core partial products.
"""

import sys
import types

import numpy as np

import concourse.bass as bass
import concourse.mybir as mybir
import concourse.tile as tile

F32 = mybir.dt.float32
AF = mybir.ActivationFunctionType
OP = mybir.AluOpType

_R = 8  # band height (output rows per band)


# ---------------------------------------------------------------------------
# Workaround: the pinned walrus rejects instructions carrying more than a
# couple of sem waits ("Too many sync wait commands", CoreV3GenImpl
# setupSyncWait). Hoist excess waits onto same-engine NOPs inserted right
# before the offending instruction.
_MAX_WAITS = 1


def _split_excess_waits(nc, max_waits=_MAX_WAITS):
    import bass_rust

    n_split = 0
    for f in nc.m.functions:
        for blk in f.blocks:
            out = []
            for inst in blk.instructions:
                si = inst.sync_info
                if si is not None and len(si.on_wait) > max_waits:
                    waits = list(si.on_wait)
                    extra, keep = waits[:-max_waits], waits[-max_waits:]
                    for i0 in range(0, len(extra), max_waits):
                        nop = mybir.InstNoOp(
                            name=f"{inst.name}_xw{i0}", ins=[], outs=[]
                        )
                        nop.engine = inst.engine
                        nop.sync_info = bass_rust.SyncInfo(
                            on_wait=extra[i0 : i0 + max_waits], on_update=[]
                        )
                        nc.register_instruction(nop)
                        out.append(nop)
                        n_split += 1
                    inst.sync_info = bass_rust.SyncInfo(
                        on_wait=keep, on_update=list(si.on_update)
                    )
                out.append(inst)
            blk.instructions = out
    return n_split


def _ensure_axon_hooks_importable():
    # bass_utils imports antenv.axon_hooks when tracing is requested; the
    # container's antenv stub lacks it. Provide a no-op registry so the
    # import never crashes (tracing then just degrades gracefully).
    try:
        import antenv.axon_hooks  # noqa: F401
    except Exception:
        try:
            import antenv

            mod = types.ModuleType("antenv.axon_hooks")
            mod._hook = None
            mod.set_axon_ntff_profile_hook = lambda h: setattr(mod, "_hook", h)
            mod.get_axon_ntff_profile_hook = lambda: mod._hook
            sys.modules["antenv.axon_hooks"] = mod
            antenv.axon_hooks = mod
        except Exception:
            pass


# ---------------------------------------------------------------------------
# Device program

# Conv operands: fp16 runs the PE at full rate (1 cyc/row, like bf16) but
# carries a 10-bit mantissa — conv error ~5e-4 vs bf16's ~3e-3. fp32r would
# be exact-ish but its fused 4-byte weight load can't pipeline.
CONV_DT = mybir.dt.float16
CHAIN_DT = mybir.dt.float16  # z/s/a/b/h/p tiles + output (host upcasts)
WPOOL_BUFS = 4
XPOOL_BUFS = 3


def build_nc(H, W, with_init_fixup=True):
    """One-core program; all 8 cores run it SPMD with different inputs."""
    R = _R
    Rh = R // 2  # rows per half-band
    RR = R + 1  # input rows resident per band (dy0/dy1 buffer)
    Wp = W + 2
    assert H % R == 0 and W % 2 == 0
    nbands = H // R
    FW = Rh * W  # free width of one half-band slab (per partition)
    cdt = CONV_DT
    wdt = CHAIN_DT

    nc = bass.Bass("TRN2", target_bir_lowering=False, debug=False)
    xp = nc.dram_tensor("xp", [64, H + 2, Wp], cdt, kind="ExternalInput").ap()
    wts = nc.dram_tensor("wts", [128, 15, 128], cdt, kind="ExternalInput").ap()
    consts = nc.dram_tensor("consts", [128, 8], F32, kind="ExternalInput").ap()
    # out free dim: [band][group][half-rows x W]; partitions = (half, chan)
    out = nc.dram_tensor("out", [128, 2 * H * W // 128 * 64], wdt,
                         kind="ExternalOutput").ap()
    # 2*H*W*64/128 = H*W: per-partition free size is nbands * 2 * FW = H*W

    with tile.TileContext(nc) as tc:
        with (
            tc.tile_pool(name="const", bufs=1) as cpool,
            tc.tile_pool(name="xin", bufs=XPOOL_BUFS) as xpool,
            tc.tile_pool(name="work", bufs=WPOOL_BUFS) as wpool,
            tc.tile_pool(name="psum", bufs=2, space="PSUM") as ppool,
        ):
            wts_sb = cpool.tile([128, 15, 128], cdt)
            nc.sync.dma_start(wts_sb[:], wts)
            cst = cpool.tile([128, 8], F32)
            nc.sync.dma_start(cst[:], consts)
            # per-group bias vectors in (half, chan) layout
            bias = [[cst[:, 3 * g + c : 3 * g + c + 1] for c in range(3)]
                    for g in range(2)]  # bias[g][conv]
            init = [cst[:, 6 + g : 7 + g] for g in range(2)]

            for band in range(nbands):
                y0 = band * R
                # x2: dy0 rows at partitions 0:64, dy1 rows at 64:128
                x2 = xpool.tile([128, RR, Wp], cdt)
                nc.sync.dma_start(x2[0:64], xp[:, y0 : y0 + RR, :])
                nc.sync.dma_start(x2[64:128], xp[:, y0 + 1 : y0 + 1 + RR, :])
                # x3: dy2 rows; lower = col+0, upper = col+1. Upper's last
                # column is never loaded; zero it so the K=128 dx2 stream
                # (zero top weights) can't hit NaN garbage.
                x3 = xpool.tile([128, R, Wp], cdt)
                nc.sync.dma_start(x3[0:64], xp[:, y0 + 2 : y0 + 2 + R, :])
                nc.vector.memset(x3[64:128, :, Wp - 1 : Wp], 0.0)
                nc.sync.dma_start(
                    x3[64:128, :, 0 : Wp - 1], xp[:, y0 + 2 : y0 + 2 + R, 1:Wp]
                )

                # per-group work tiles, all in (half, chan) partition layout;
                # free dim = Rh rows x W cols, row-major
                z_t = [wpool.tile([128, FW], wdt, name=f"z{g}") for g in range(2)]
                s_t = [wpool.tile([128, FW], wdt, name=f"s{g}") for g in range(2)]
                # [a | b] contiguous so the scan reads one tile
                ab_t = [wpool.tile([128, 2 * FW], wdt, name=f"ab{g}") for g in range(2)]
                h_t = [wpool.tile([128, FW], wdt, name=f"h{g}") for g in range(2)]
                # p for both groups in one tile -> single out DMA per band
                p_t = wpool.tile([128, 2 * FW], wdt)

                for j in range(2):  # j covers rows (2j, 2j+1) of each half
                    sl = slice(j * 2 * W, (j + 1) * 2 * W)
                    for g in range(2):  # scan group (fwd / bwd)
                        m0, m1 = 64 * g, 64 * g + 64
                        us = []
                        for c in range(3):  # z, h, s convs
                            u = ppool.tile(
                                [128, 2 * W], F32, name=f"u{c}g{g}",
                                tag=f"u{c}g{g}", bufs=(2 if c == 1 else 1),
                            )
                            for hh in range(2):  # concurrent col-group halves
                                r0 = hh * Rh + 2 * j
                                p0, p1 = 64 * hh, 64 * hh + 64
                                for dx in range(3):  # dy0+dy1 pairs
                                    nc.tensor.matmul(
                                        u[p0:p1],
                                        wts_sb[:, 3 * c + dx, m0:m1],
                                        x2[:, r0 : r0 + 2, dx : dx + W],
                                        start=(dx == 0),
                                        stop=False,
                                    )
                                # dy2 (dx0, dx1) pair
                                nc.tensor.matmul(
                                    u[p0:p1],
                                    wts_sb[:, 9 + c, m0:m1],
                                    x3[:, r0 : r0 + 2, 0:W],
                                    start=False,
                                    stop=False,
                                )
                                # dy2 dx2 (upper weights zero -> K=128 uniform)
                                nc.tensor.matmul(
                                    u[p0:p1],
                                    wts_sb[:, 12 + c, m0:m1],
                                    x3[:, r0 : r0 + 2, 2 : 2 + W],
                                    start=False,
                                    stop=True,
                                )
                            us.append(u)
                        nc.scalar.activation(
                            z_t[g][:, sl], us[0][:], AF.Sigmoid, bias=bias[g][0]
                        )
                        nc.scalar.activation(
                            s_t[g][:, sl], us[2][:], AF.Sigmoid, bias=bias[g][2]
                        )
                        # b = (u_h + bias_h) * z
                        nc.vector.scalar_tensor_tensor(
                            ab_t[g][:, FW + j * 2 * W : FW + (j + 1) * 2 * W],
                            us[1][:], bias[g][1], z_t[g][:, sl],
                            op0=OP.add, op1=OP.mult,
                        )

                for g in range(2):
                    a_f = ab_t[g][:, 0:FW]
                    b_f = ab_t[g][:, FW : 2 * FW]
                    # a = 1 - z (on ACT: Identity(-z + 1))
                    nc.scalar.activation(
                        a_f, z_t[g][:], AF.Identity, bias=1.0, scale=-1.0
                    )
                    a3 = a_f.rearrange("p (r w) -> p r w", w=W)
                    b3 = b_f.rearrange("p (r w) -> p r w", w=W)
                    edge = 0 if g == 0 else W - 1
                    # fold the (normally zero) scan init into b at each row
                    # edge, then zero `a` there so the flat scan restarts
                    # per row.
                    if with_init_fixup:
                        nc.vector.scalar_tensor_tensor(
                            b3[:, :, edge], a3[:, :, edge], init[g],
                            b3[:, :, edge], op0=OP.mult, op1=OP.add,
                        )
                    nc.scalar.activation(
                        a3[:, :, edge], a3[:, :, edge], AF.Copy,
                        bias=0.0, scale=0.0,
                    )
                    # scan at full 128-partition width; group 1 scans
                    # backward via reversed APs
                    if g == 0:
                        nc.vector.tensor_tensor_scan(
                            h_t[g][:, :], a_f, b_f, 0.0,
                            op0=OP.mult, op1=OP.add,
                        )
                    else:
                        nc.vector.tensor_tensor_scan(
                            h_t[g][:, ::-1], a_f[:, ::-1], b_f[:, ::-1], 0.0,
                            op0=OP.mult, op1=OP.add,
                        )
                    nc.vector.tensor_mul(
                        p_t[:, g * FW : (g + 1) * FW], s_t[g][:], h_t[g][:]
                    )
                # out store on the (otherwise idle) gpsimd software-DGE
                # queue: keeps the sync queue free of head-of-line blocking
                # behind the scan chain.
                nc.gpsimd.dma_start(
                    out[:, band * 2 * FW : (band + 1) * 2 * FW], p_t[:]
                )
    _split_excess_waits(nc)
    return nc


# ---------------------------------------------------------------------------
# Host side

_NC_CACHE = {}


def _get_nc(H, W, with_init_fixup=True):
    key = (H, W, with_init_fixup)
    if key not in _NC_CACHE:
        _NC_CACHE[key] = build_nc(H, W, with_init_fixup)
    return _NC_CACHE[key]


def make_in_maps(inputs, H, W):
    """Build the 8 per-core input dicts from the full problem inputs."""
    xs = np.ascontiguousarray(np.asarray(inputs["xs"], dtype=np.float32))
    B = xs.shape[0]
    Ws, Bs = {}, {}
    for tag in ("z", "h", "s"):
        w = np.asarray(inputs["w_" + tag], dtype=np.float32)
        g = np.asarray(inputs["g_" + tag], dtype=np.float32)
        be = np.asarray(inputs["b_" + tag], dtype=np.float32)
        m = np.asarray(inputs["m_" + tag], dtype=np.float32)
        v = np.asarray(inputs["v_" + tag], dtype=np.float32)
        inv = g / np.sqrt(v + 1e-5)
        Ws[tag] = w * inv[:, None, None, None]
        Bs[tag] = be - m * inv
    init = {
        k: np.asarray(inputs[k], dtype=np.float32).reshape(-1)
        for k in ("h20", "h21", "h30", "h31")
    }

    in_maps = []
    for b in range(B):
        for orient in (0, 1):
            if orient == 0:
                img = xs[b]
                ch = slice(128, 256)
                init_a, init_b = init["h30"], init["h31"]
            else:
                img = xs[b].transpose(0, 2, 1)
                ch = slice(0, 128)
                init_a, init_b = init["h20"], init["h21"]
            xpad = np.pad(img, ((0, 0), (1, 1), (1, 1)))
            wts = np.zeros((128, 15, 128), np.float32)
            consts = np.zeros((128, 8), np.float32)
            for c, tag in enumerate(("z", "h", "s")):
                wc = Ws[tag][ch]  # (128, 64, 3, 3) [cout, cin, ky, kx]
                if orient == 1:
                    wc = wc.transpose(0, 1, 3, 2)
                for dx in range(3):
                    wts[0:64, 3 * c + dx, :] = wc[:, :, 0, dx].T
                    wts[64:128, 3 * c + dx, :] = wc[:, :, 1, dx].T
                wts[0:64, 9 + c, :] = wc[:, :, 2, 0].T
                wts[64:128, 9 + c, :] = wc[:, :, 2, 1].T
                wts[0:64, 12 + c, :] = wc[:, :, 2, 2].T
                # biases in (half, chan) layout, separate per scan group
                bg = Bs[tag][ch]
                consts[0:64, c] = bg[0:64]
                consts[64:128, c] = bg[0:64]
                consts[0:64, 3 + c] = bg[64:128]
                consts[64:128, 3 + c] = bg[64:128]
            consts[0:64, 6] = init_a
            consts[64:128, 6] = init_a
            consts[0:64, 7] = init_b
            consts[64:128, 7] = init_b
            cnp = mybir.dt.np(CONV_DT)
            if xpad.dtype != cnp:
                xpad = xpad.astype(cnp)
                wts = wts.astype(cnp)
            in_maps.append(
                {
                    "xp": np.ascontiguousarray(xpad),
                    "wts": wts,
                    "consts": consts,
                }
            )
    return in_maps


def gather_output(core_outs, B, H, W):
    """core_outs: list of 8 arrays (128, H*W) in core order (b-major).

    Device layout: partitions = (half hh in {0,1}, chan c in 0..63);
    free = [band][group g in {0,1}][row r in 0..Rh-1][col]. Global row of
    an element = band*R + hh*Rh + r.
    """
    R, Rh = _R, _R // 2
    nb = H // R
    out = np.empty((B, 64, H, W), np.float32)
    for b in range(B):
        for orient in (0, 1):
            o = core_outs[2 * b + orient].astype(np.float32)
            o = o.reshape(2, 64, nb, 2, Rh, W)  # hh, c, band, g, r, w
            o = o.sum(axis=3)  # sum the two scan groups: hh, c, band, r, w
            o = o.transpose(1, 2, 0, 3, 4).reshape(64, H, W)
            if orient == 0:
                out[b] = o
            else:
                out[b] += o.transpose(0, 2, 1)
    return out


def kernel(**inputs):
    from concourse.bass_utils import run_bass_kernel_spmd

    _ensure_axon_hooks_importable()
    xs = inputs["xs"]
    B, C, H, W = xs.shape
    # the scan-init fixup ops are only needed for nonzero initial states
    # (the problem spec ships all-zero inits)
    need_fixup = any(
        np.any(np.asarray(inputs[k], dtype=np.float32))
        for k in ("h20", "h21", "h30", "h31")
    )
    nc = _get_nc(H, W, with_init_fixup=need_fixup)
    in_maps = make_in_maps(inputs, H, W)
    res = run_bass_kernel_spmd(nc, in_maps, core_ids=list(range(len(in_maps))))
    outs = [res.results[c]["out"] for c in range(len(in_maps))]
    return gather_output(outs, B, H, W)
